# revision 7
# baseline (speedup 1.0000x reference)
"""DMGCGRUCell Trainium2 kernel v2: 8-core SPMD batch sharding, fp8 GCN.

- An/hw/H/a1w in fp8 e4m3 (host power-of-2 scales); GCN aggregate and z-matmul
  use fp8 DoubleRow (0.5 cyc/row, K=256/matmul): 4x less PE than bf16, 2x less
  AnT HBM traffic.
- (b,e)-packed 128-partition layout for attention: relu/exp/recip/combine all
  at full partition width (TRN2 engine cost is free-size only).
- Softmax normalization at the end: S = (sum_g aU_g*H_g) * bcast(1/Z).
- Cross-region software pipeline: region i u/r passes overlap region i-1
  candidate pass + GRU; the three attention chains (u,r,c-prev) advance
  stage-interleaved so PE/Act/DVE/Pool overlap.
- DMAs batched (13/region) to respect the serialized HWDGE unit.
"""
import numpy as np
import concourse.bass as bass
import concourse.tile as tile
from concourse import bacc, mybir
from concourse.bass_utils import run_bass_kernel_spmd

B, N, R, S, G = 16, 10000, 10, 1000, 4
DIN, DH = 32, 64
NCORES = 8
BL = B // NCORES      # 2 local batches
SP = 1024             # padded s (output rows per region)
TP = 1024             # padded t (contraction dim)
NKT = TP // 128       # 8 t-chunks
W2 = BL * SP          # 2048 (b, s) columns
F32 = mybir.dt.float32
BF16 = mybir.dt.bfloat16
FP8 = mybir.dt.float8e4
AF = mybir.ActivationFunctionType
ALU = mybir.AluOpType
DR = mybir.MatmulPerfMode.DoubleRow
BIAS_W = np.array([0.1, 0.1, 0.1, 1.0], dtype=np.float32)

SA_LOG, SHW_LOG, SH_LOG, SA1_LOG = 15, 6, 4, 6
SA = float(2.0 ** SA_LOG)
SHW = float(2.0 ** SHW_LOG)
SH = float(2.0 ** SH_LOG)
SA1 = float(2.0 ** SA1_LOG)
H_DESCALE = float(2.0 ** -(SA_LOG + SHW_LOG - SH_LOG))   # psum*this = H*2^4
Z_DESCALE = float(2.0 ** -(SH_LOG + SA1_LOG))            # zp*this = z
M_DESCALE = float(2.0 ** -SH_LOG)                        # Hq*this = H

_cache = {}


def _build():
    nc = bacc.Bacc("TRN2", target_bir_lowering=False, debug=False, num_devices=NCORES)

    def din(name, shape, dt=BF16):
        return nc.dram_tensor(name, list(shape), dt, kind="ExternalInput").ap()

    xhT = din("xhT", (BL, R, 96, TP))          # rows: 64 h_prev, 32 x_t
    An8 = din("An8", (G, R, NKT, 128, SP), FP8)
    rsB = din("rsB", (R, 4, W2))               # rows 0-1: (rs0,rs1) x (b,s);
                                               # rows 2-3: rs1 per b (first SP)
    Wur = din("Wur", (96, 512))
    Wc = din("Wc", (96, 256))
    Wch64 = din("Wch64", (128, 256))        # rows 64:128 = W_h
    Wcx = din("Wcx", (32, 256))             # W_x
    a1q = {k: din(f"a1q_{k}", (128, G, 64), FP8) for k in "urc"}
    a1rs = {k: din(f"a1rs_{k}", (2, 64)) for k in "urc"}
    a1b2 = {k: din(f"a1b2_{k}", (128, 1), F32) for k in "urc"}
    a2wb2 = {k: din(f"a2wb2_{k}", (128, 8)) for k in "urc"}
    lgis = din("lgis", (2, 8))
    selab = din("selab", (8, G * 128))
    selsm = din("selsm", (8, 2))
    selrb = din("selrb", (2, 128))
    identb = din("identb", (128, 64))
    out_l = nc.dram_tensor("out_l", [BL, R, S, 64], F32, kind="ExternalOutput").ap()

    with tile.TileContext(nc, trace_sim=False) as tc:
        import contextlib
        ctx = contextlib.ExitStack()
        with ctx, nc.allow_low_precision(reason="fp8/bf16 data; f32 PSUM accum"):
            const = ctx.enter_context(tc.tile_pool(name="const", bufs=1))
            sb1 = ctx.enter_context(tc.tile_pool(name="sb1", bufs=1))
            sb2 = ctx.enter_context(tc.tile_pool(name="sb2", bufs=2))
            psB = ctx.enter_context(tc.tile_pool(name="psB", bufs=2, space="PSUM"))
            psA = ctx.enter_context(tc.tile_pool(name="psA", bufs=2, space="PSUM"))

            def cload(name, src, shape, dt=BF16):
                t = const.tile(list(shape), dt, tag=name, name=name)
                nc.sync.dma_start(t[:], src)
                return t

            wur_t = cload("wur", Wur[:], (96, 512))
            wc_t = cload("wc", Wc[:], (96, 256))
            wch64_t = cload("wch64", Wch64[:], (128, 256))
            wcx_t = cload("wcx", Wcx[:], (32, 256))
            lgis_t = cload("lgis", lgis[:], (2, 8))
            selab_t = cload("selab", selab[:], (8, G * 128))
            selsm_t = cload("selsm", selsm[:], (8, 2))
            selrb_t = cload("selrb", selrb[:], (2, 128))
            id_t = cload("identb", identb[:], (128, 64))
            a1q_t, a1rs_t, a1b2_t, a2wb2_t = {}, {}, {}, {}
            for k in "urc":
                a1q_t[k] = cload(f"a1q{k}", a1q[k][:], (128, G, 64), FP8)
                a1rs_t[k] = cload(f"a1rs{k}", a1rs[k][:], (2, 64))
                bb = const.tile([128, 1], F32, tag=f"a1b2{k}", name=f"a1b2{k}")
                nc.sync.dma_start(bb[:], a1b2[k][:])
                a1b2_t[k] = bb
                a2wb2_t[k] = cload(f"a2wb2{k}", a2wb2[k][:], (128, 8))

            # ---------------- phase helpers ----------------
            def dma_in(r):
                d = {}
                inpT1, inpT2 = [], []
                for b in range(BL):
                    t1 = sb2.tile([96, TP], BF16, tag=f"inp1{b}", name=f"inp1{b}")
                    nc.sync.dma_start(t1[:], xhT[b, r])
                    inpT1.append(t1)
                    t2 = sb2.tile([32, TP], BF16, tag=f"inp2{b}", name=f"inp2{b}")
                    nc.sync.dma_start(t2[:], xhT[b, r, 64:96])
                    inpT2.append(t2)
                d["inpT1"], d["inpT2"] = inpT1, inpT2
                hpP = sb2.tile([128, TP], BF16, tag="hpP", name="hpP")
                nc.sync.dma_start(hpP[:], xhT[:, r, 0:64, :])
                d["hpP"] = hpP
                rsB_t = sb2.tile([2, W2], BF16, tag="rsB", name="rsB")
                nc.sync.dma_start(rsB_t[:], rsB[r, 0:2])
                d["rsB"] = rsB_t
                ig = sb2.tile([2, SP], BF16, tag="ig", name="ig")
                nc.sync.dma_start(ig[:], rsB[r, 2:4, 0:SP])
                d["ig"] = ig
                return d

            def dma_an(r):
                an = []
                for g in range(G):
                    a = sb2.tile([128, NKT, SP], FP8, tag=f"an{g}", name=f"an{g}")
                    nc.sync.dma_start(a[:, :, :],
                                      An8[g, r].rearrange("k p s -> p k s"))
                    an.append(a)
                return an

            def mm1(pairs_per_b, hwq, ncols, esplit):
                """hwq[:, tcd, (q,b,e)] = fp8(SHW * inp_b^T W).

                pairs_per_b[b] = [(lhs_fn, rhs_ap), ...] accumulated into one
                psum region (lhs/rhs share a base partition per pair)."""
                nq = ncols // 64
                for tcd in range(NKT):
                    psf = psB.tile([128, 1024], F32, tag="big", name="mm1ps")
                    ps = psf[:, 0:BL * ncols]
                    for b in range(BL):
                        pairs = pairs_per_b[b]
                        for pi, (lhs_fn, rhs_ap) in enumerate(pairs):
                            nc.tensor.matmul(ps[:, b * ncols:(b + 1) * ncols],
                                             lhs_fn(tcd), rhs_ap,
                                             start=(pi == 0),
                                             stop=(pi == len(pairs) - 1),
                                             skip_group_check=True)
                    dst = hwq[:, tcd, :].rearrange("p (q b e) -> p b q e", b=BL, e=64)
                    src = ps.rearrange("p (b q e) -> p b q e", q=nq, e=64)
                    if esplit[tcd] == 1:
                        nc.scalar.activation(dst, src, AF.Copy, scale=SHW)
                    else:
                        nc.vector.tensor_scalar_mul(dst, src, SHW)

            def mm2(blk, an, hwq, col0, Hq):
                """GCN aggregate fp8 DoubleRow; relu -> Hq fp8 (x 2^4)."""
                for g in range(G):
                    pss = psB.tile([128, SP], F32, tag="big", name="mm2")
                    for kp in range(NKT // 2):
                        lhs = hwq[:, 2 * kp:2 * kp + 2,
                                  col0 + g * 128:col0 + (g + 1) * 128]
                        for sc in range(2):
                            nc.tensor.matmul(
                                pss[:, sc * 512:(sc + 1) * 512], lhs,
                                an[g][:, 2 * kp:2 * kp + 2, sc * 512:(sc + 1) * 512],
                                start=(kp == 0), stop=(kp == NKT // 2 - 1),
                                perf_mode=DR)
                    if blk == "c" and g % 2 == 1:
                        nc.vector.tensor_scalar(Hq[g // 2][:, g % 2, :], pss[:],
                                                H_DESCALE, 0.0,
                                                op0=ALU.mult, op1=ALU.max)
                    else:
                        nc.scalar.activation(Hq[g // 2][:, g % 2, :], pss[:],
                                             AF.Relu, scale=H_DESCALE)

            # ---- staged attention: ctx dict per block, stages interleaved
            def att_z(c):
                blk = c["blk"]
                zps = []
                for b in range(BL):
                    rows = slice(b * 64, (b + 1) * 64)
                    zp = psA.tile([64, SP], F32, tag="att", name=f"zp{blk}{b}")
                    for sh in range(2):
                        cs = slice(sh * 512, (sh + 1) * 512)
                        for gp in range(2):
                            nc.tensor.matmul(zp[:, cs],
                                             a1q_t[blk][rows, 2 * gp:2 * gp + 2, :],
                                             c["Hq"][gp][rows, :, cs],
                                             start=(gp == 0), stop=False,
                                             perf_mode=DR, skip_group_check=True)
                        nc.tensor.matmul(
                            zp[:, cs], a1rs_t[blk][:],
                            c["rsB"][0:2, b * SP + sh * 512:b * SP + (sh + 1) * 512],
                            start=False, stop=True, skip_group_check=True)
                    zps.append(zp)
                c["zps"] = zps

            def att_zrelu(c):
                blk = c["blk"]
                zS = sb2.tile([128, SP], BF16, tag=f"zS{blk}", name=f"zS{blk}")
                for b in range(BL):
                    rows = slice(b * 64, (b + 1) * 64)
                    nc.scalar.activation(zS[rows, :], c["zps"][b][:], AF.Relu,
                                         scale=Z_DESCALE,
                                         bias=a1b2_t[blk][rows, :])
                c["zS"] = zS

            def att_lg(c):
                blk = c["blk"]
                lgp = psA.tile([8, SP], F32, tag="att", name=f"lgp{blk}")
                for sh in range(2):
                    cs = slice(sh * 512, (sh + 1) * 512)
                    nc.tensor.matmul(lgp[:, cs], a2wb2_t[blk][:], c["zS"][:, cs],
                                     start=True, stop=False)
                    nc.tensor.matmul(lgp[:, cs], lgis_t[:],
                                     c["ig"][:, sh * 512:(sh + 1) * 512],
                                     start=False, stop=True)
                c["lgp"] = lgp

            def att_exp(c):
                blk = c["blk"]
                aU = sb2.tile([8, SP], BF16, tag=f"aU{blk}", name=f"aU{blk}")
                nc.scalar.activation(aU[:], c["lgp"][:], AF.Exp)
                c["aU"] = aU

            def att_sm(c):
                blk = c["blk"]
                smp = psA.tile([2, SP], F32, tag="att", name=f"smp{blk}")
                for sh in range(2):
                    cs = slice(sh * 512, (sh + 1) * 512)
                    nc.tensor.matmul(smp[:, cs], selsm_t[:], c["aU"][:, cs],
                                     start=True, stop=True)
                c["smp"] = smp

            def att_recip(c):
                blk = c["blk"]
                recS = sb2.tile([2, SP], BF16, tag=f"rec{blk}", name=f"rec{blk}",
                                bufs=1)
                nc.vector.reciprocal(recS[:], c["smp"][:])
                c["recS"] = recS

            def att_comb(c):
                blk = c["blk"]
                mt = []
                for gp in range(2):
                    mtag = f"mu{gp}" if blk == "u" else f"mx{gp}"
                    m = sb2.tile([128, 2, SP], BF16, tag=mtag,
                                 name=f"m{blk}{gp}", bufs=1)
                    for sh in range(2):
                        ab = psA.tile([128, SP], F32, tag="att", name=f"ab{blk}")
                        for gi in range(2):
                            g = 2 * gp + gi
                            nc.tensor.matmul(ab[:, gi * 512:(gi + 1) * 512],
                                             selab_t[:, g * 128:(g + 1) * 128],
                                             c["aU"][:, sh * 512:(sh + 1) * 512],
                                             start=True, stop=True)
                        nc.vector.scalar_tensor_tensor(
                            m[:, :, sh * 512:sh * 512 + 512],
                            c["Hq"][gp][:, :, sh * 512:sh * 512 + 512], M_DESCALE,
                            ab[:, :].rearrange("p (gi s) -> p gi s", gi=2),
                            op0=ALU.mult, op1=ALU.mult)
                    mt.append(m)
                c["mt"] = mt

            def att_adds(c):
                blk = c["blk"]
                mt = c["mt"]
                tA = sb2.tile([128, SP], BF16, tag="tAx", name=f"tA{blk}",
                              bufs=1)
                nc.gpsimd.tensor_add(tA[:], mt[0][:, 0, :], mt[0][:, 1, :])
                tB = sb2.tile([128, SP], BF16, tag="tBx", name=f"tB{blk}",
                              bufs=1)
                nc.vector.tensor_add(tB[:], mt[1][:, 0, :], mt[1][:, 1, :])
                mS = sb2.tile([128, SP], BF16, tag=f"mS{blk}", name=f"mS{blk}",
                              bufs=1)
                nc.gpsimd.tensor_add(mS[:], tA[:], tB[:])
                c["mS"] = mS

            def att_rb(c):
                blk = c["blk"]
                rbp = psA.tile([128, SP], F32, tag="att", name=f"rbp{blk}")
                for sh in range(2):
                    cs = slice(sh * 512, (sh + 1) * 512)
                    nc.tensor.matmul(rbp[:, cs], selrb_t[:], c["recS"][:, cs],
                                     start=True, stop=True)
                c["rbp"] = rbp

            STAGES = [att_z, att_zrelu, att_lg, att_exp, att_sm, att_recip,
                      att_comb, att_adds, att_rb]

            # ---------------- pipelined region loop ----------------
            st_prev = {}
            an_pending = None
            for i in range(R + 1):
                cur = i if i < R else None
                prev = i - 1 if i >= 1 else None

                cu = cr = cc = None
                if cur is not None:
                    d = dma_in(cur)
                    d["an"] = an_pending if cur > 0 else dma_an(0)
                    hwq1 = sb1.tile([128, NKT, 1024], FP8, tag="hwq1", name="hwq1")
                    mm1([[(lambda tcd, t=d["inpT1"][b]:
                           t[:, tcd * 128:(tcd + 1) * 128], wur_t[:])]
                         for b in range(BL)], hwq1, 512,
                        [1, 1, 1, 1, 1, 1, 1, 1])
                    Hqur = {blk: [sb1.tile([128, 2, SP], FP8, tag=f"Hq{blk}{gp}",
                                           name=f"Hq{blk}{gp}") for gp in range(2)]
                            for blk in ("u", "r")}
                    cu = dict(blk="u", Hq=Hqur["u"], rsB=d["rsB"], ig=d["ig"])
                    cr = dict(blk="r", Hq=Hqur["r"], rsB=d["rsB"], ig=d["ig"])
                    mm2("u", d["an"], hwq1, 0, Hqur["u"])
                if prev is not None:
                    hwq2 = sb1.tile([128, NKT, 512], FP8, tag="hwq2", name="hwq2")
                    it2, rhP = st_prev["inpT2"], st_prev["rhP"]
                    mm1([[(lambda tcd: rhP[0:64, tcd * 128:(tcd + 1) * 128],
                           wc_t[0:64, :]),
                          (lambda tcd, t=it2[0]: t[:, tcd * 128:(tcd + 1) * 128],
                           wcx_t[:])],
                         [(lambda tcd: rhP[64:128, tcd * 128:(tcd + 1) * 128],
                           wch64_t[64:128, :]),
                          (lambda tcd, t=it2[1]: t[:, tcd * 128:(tcd + 1) * 128],
                           wcx_t[:])]], hwq2, 256,
                        [1, 1, 1, 1, 1, 1, 1, 1])
                    Hqc = [sb1.tile([128, 2, SP], FP8, tag=f"Hqc{gp}",
                                    name=f"Hqc{gp}") for gp in range(2)]
                    cc = dict(blk="c", Hq=Hqc, rsB=st_prev["rsB"],
                              ig=st_prev["ig"])
                if cur is not None:
                    mm2("r", d["an"], hwq1, 512, Hqur["r"])
                    att_z(cu); att_zrelu(cu)
                    att_z(cr); att_zrelu(cr)
                    att_lg(cu); att_exp(cu)
                if prev is not None:
                    mm2("c", st_prev["an"], hwq2, 0, Hqc)
                if cur is not None and cur + 1 < R:
                    an_pending = dma_an(cur + 1)
                if cur is not None:
                    att_lg(cr); att_exp(cr)
                    att_sm(cu); att_recip(cu)
                    att_sm(cr); att_recip(cr)
                    att_comb(cu)
                if prev is not None:
                    att_z(cc); att_zrelu(cc)
                if cur is not None:
                    att_comb(cr)
                if prev is not None:
                    att_lg(cc); att_exp(cc)
                if cur is not None:
                    att_adds(cu); att_rb(cu)
                if prev is not None:
                    att_sm(cc); att_recip(cc)
                if cur is not None:
                    att_adds(cr); att_rb(cr)
                if prev is not None:
                    att_comb(cc)
                    att_adds(cc); att_rb(cc)

                # consumes; sigmoid via tanh:
                # u = (tanh(S_u/2)+1)/2 ; GRU: d = 0.5*((th+hp) + tanh*(th-hp))
                if cur is not None:
                    Su = sb2.tile([128, SP], BF16, tag="Su", name="Su", bufs=1)
                    nc.vector.tensor_mul(Su[:], cu["mS"][:], cu["rbp"][:])
                    uT = sb2.tile([128, SP], BF16, tag="uT", name="uT")
                    nc.scalar.activation(uT[:], Su[:], AF.Tanh, scale=0.5)
                    d["uT"] = uT
                    Sr = sb2.tile([128, SP], BF16, tag="Sr", name="Sr", bufs=1)
                    nc.vector.tensor_mul(Sr[:], cr["mS"][:], cr["rbp"][:])
                    rhP = sb2.tile([128, TP], BF16, tag="rhP", name="rhP")
                    nc.gpsimd.tensor_mul(rhP[:], Sr[:], d["hpP"][:])
                    d["rhP"] = rhP
                if prev is not None:
                    c = cc
                    th = sb2.tile([128, SP], BF16, tag="th", name="th", bufs=1)
                    nc.vector.tensor_mul(c["mS"][:], c["mS"][:], c["rbp"][:])
                    nc.scalar.activation(th[:], c["mS"][:], AF.Tanh)
                    hpP, uTp = st_prev["hpP"], st_prev["uT"]
                    sm_ = sb2.tile([128, SP], BF16, tag="sm_", name="sm_", bufs=1)
                    nc.gpsimd.tensor_add(sm_[:], th[:], hpP[:])
                    df = sb2.tile([128, SP], BF16, tag="df", name="df", bufs=1)
                    nc.vector.tensor_sub(df[:], th[:], hpP[:])
                    # df = 0.5*u_tanh*(th-hp); dd = 0.5*(th+hp) + df
                    nc.vector.scalar_tensor_tensor(df[:], uTp[:], 0.5, df[:],
                                                   op0=ALU.mult, op1=ALU.mult)
                    dd = sb2.tile([128, SP], BF16, tag="dd", name="dd", bufs=1)
                    nc.vector.scalar_tensor_tensor(dd[:], sm_[:], 0.5, df[:],
                                                   op0=ALU.mult, op1=ALU.add)
                    for b in range(BL):
                            tp = psA.tile([128, 512], BF16, tag="att", name="tp")
                            for ci in range(8):
                                nc.tensor.transpose(
                                    tp[:, ci * 64:(ci + 1) * 64],
                                    dd[b * 64:(b + 1) * 64,
                                       ci * 128:(ci + 1) * 128],
                                    id_t[b * 64:(b + 1) * 64, :])
                            gs = sb2.tile([128, 512], F32, tag=f"gs{b}",
                                          name=f"gs{b}", bufs=1)
                            nc.vector.tensor_copy(gs[:], tp[:])
                            dst1 = out_l[b, prev, 0:896, :].rearrange(
                                "(c p) e -> p c e", p=128)
                            src1 = gs[:, 0:448].rearrange("p (c e) -> p c e", c=7)
                            nc.sync.dma_start(dst1, src1)
                            nc.sync.dma_start(out_l[b, prev, 896:1000, :],
                                              gs[0:104, 448:512])

                if cur is not None:
                    st_prev = d

    nc.compile()
    return nc


def _prep(inputs):
    import ml_dtypes
    bf = ml_dtypes.bfloat16
    f8 = ml_dtypes.float8_e4m3

    A = np.asarray(inputs["A"], np.float32)
    deg = np.clip(A.sum(-1), 1e-5, None) ** -0.5
    An_f = deg[..., :, None] * (A + np.eye(S, dtype=np.float32)) * deg[..., None, :]
    AnT = np.zeros((G, R, TP, SP), np.float32)
    AnT[:, :, 0:S, 0:S] = An_f.transpose(0, 1, 3, 2) * SA
    An8 = AnT.reshape(G, R, NKT, 128, SP).astype(f8)

    x_t = np.asarray(inputs["x_t"], np.float32).reshape(B, R, S, DIN)
    h_prev = np.asarray(inputs["h_prev"], np.float32).reshape(B, R, S, DH)
    rs = np.asarray(inputs["resid_stats"], np.float32).reshape(B, R, S, 2)
    xhT = np.zeros((B, R, 96, TP), bf)
    xhT[:, :, 0:64, 0:S] = h_prev.transpose(0, 1, 3, 2)
    xhT[:, :, 64:96, 0:S] = x_t.transpose(0, 1, 3, 2)

    perm = np.concatenate([np.arange(32, 96), np.arange(0, 32)])
    Wur = np.concatenate([inputs["W_u"].transpose(1, 0, 2).reshape(96, 256),
                          inputs["W_r"].transpose(1, 0, 2).reshape(96, 256)],
                         axis=1)[perm]
    Wc_u = inputs["W_c"].transpose(1, 0, 2).reshape(96, 256)   # rows (x32, h64)
    Wc = np.ascontiguousarray(Wc_u[perm])
    Wch64 = np.zeros((128, 256), np.float32)
    Wch64[64:128] = Wc_u[32:96]
    Wcx = np.ascontiguousarray(Wc_u[0:32])
    log1p_bw = np.log1p(BIAS_W)

    selab = np.zeros((8, G * 128), np.float32)
    for g in range(G):
        for p in range(128):
            selab[(p // 64) * 4 + g, g * 128 + p] = 1.0
    selsm = np.zeros((8, 2), np.float32)
    for row in range(8):
        selsm[row, row // 4] = 1.0
    selrb = np.zeros((2, 128), np.float32)
    for p in range(128):
        selrb[p // 64, p] = 1.0
    lgis = np.zeros((2, 8), np.float32)
    for b in range(2):
        lgis[b, b * 4:(b + 1) * 4] = log1p_bw

    common = {
        "Wur": np.ascontiguousarray(Wur).astype(bf),
        "Wc": Wc.astype(bf),
        "Wch64": Wch64.astype(bf),
        "Wcx": Wcx.astype(bf),
        "identb": np.tile(np.eye(64, dtype=np.float32), (2, 1)).astype(bf),
        "selab": selab.astype(bf),
        "selsm": selsm.astype(bf),
        "selrb": selrb.astype(bf),
        "lgis": lgis.astype(bf),
    }
    for k in "urc":
        a1w = np.asarray(inputs[f"a1w_{k}"], np.float32)
        a1q_h = np.ascontiguousarray(
            (a1w[0:256] * SA1).reshape(G, 64, 64).transpose(1, 0, 2)).astype(f8)
        common[f"a1q_{k}"] = np.concatenate([a1q_h, a1q_h], axis=0)
        common[f"a1rs_{k}"] = (a1w[256:258] * (SH * SA1)).astype(bf)
        a1b = np.asarray(inputs[f"a1b_{k}"], np.float32)
        common[f"a1b2_{k}"] = np.tile(a1b.reshape(64, 1), (2, 1)).astype(np.float32)
        a2w = np.asarray(inputs[f"a2w_{k}"], np.float32)
        a2wb2 = np.zeros((128, 8), np.float32)
        a2wb2[0:64, 0:4] = a2w
        a2wb2[64:128, 4:8] = a2w
        common[f"a2wb2_{k}"] = a2wb2.astype(bf)

    in_maps = []
    for core in range(NCORES):
        bs = slice(core * BL, (core + 1) * BL)
        rsB_c = np.zeros((R, 4, W2), bf)
        for b in range(BL):
            rsB_c[:, 0:2, b * SP:b * SP + S] = rs[core * BL + b].transpose(0, 2, 1)
            rsB_c[:, 2 + b, 0:S] = (rs[core * BL + b, :, :, 1] > 0.5)
        m = dict(common)
        m["xhT"] = np.ascontiguousarray(xhT[bs])
        m["An8"] = An8
        m["rsB"] = rsB_c
        in_maps.append(m)
    return in_maps


def kernel(**inputs) -> np.ndarray:
    if "nc" not in _cache:
        _cache["nc"] = _build()
    nc = _cache["nc"]
    in_maps = _prep(inputs)
    res = run_bass_kernel_spmd(nc, in_maps, list(range(NCORES)))
    out = np.zeros((B, R, S, DH), np.float32)
    for core in range(NCORES):
        out[core * BL:(core + 1) * BL] = res.results[core]["out_l"]
    return out.reshape(B, N, DH)
